# revision 58
# baseline (speedup 1.0000x reference)
"""Trainium2 distributed kernel for a transformer decoder layer (8 NeuronCores).

Layer: x -> LN1 -> causal MHA (16 heads, d=64) -> +res -> LN2 -> FFN(GELU) -> +res
hidden_states [2, 2048, 1024], HID=1024, FFN=4096, f32 I/O, bf16 matmul compute.

Sharding: 2 batch groups x 4 cores. Core c: batch g=c//4, rank r=c%4, owns
token rows [r*512,(r+1)*512): LN1, QKV projections, o_proj, LN2, FFN, output.
Attention is head-sharded across all 8 cores via AllToAll (uniform SPMD
program: every core runs full-sequence causal attention for compile-time-fixed
head slots {2c, 2c+1} x both batches; the A2A routes each core its heads).

Collective plan (the CC engine serializes collectives, and the one-time
collective-network init lands 25-90us into the run -- lazily, at the first
trigger):
  warmup A2A (tiny, first instruction) starts the init at t~0 so it overlaps
  LN1 + the projections instead of a real wire.
  A2A(K+Q): one combined collective (chunk j = [K^T f=j | Q^T f=j] of my 512
  tokens, f = head-pair) fired right after the K/Q projections.
  A2A(V): token-major V in one collective right behind KQ (a token-half
  split measured even: the extra mesh cost what the earlier first half
  saved -- the software-pipelined PV backlog absorbs V's arrival anyway).
  A2A #2a/2b: normalized attention output O^T routed back token-major, in
  token halves so o_proj of tokens 0:256 overlaps #2b.
DMA discipline: every dma_start costs ~0.6us of CONFIG time on its issuing
engine's sequencer, so scatter/gathers use single multi-dim-AP DMAs (max 3
dims after the partition dim).  GPSIMD-issued DMAs are NOT a shortcut: the
cost model's 25ns config figure is wrong on this stack -- measured runs
regressed ~90us (software descriptor generation on the DSP); keep bulk DMAs
on nc.sync/nc.scalar.

Attention: scores keys-on-partitions (S^T = K_h @ Q_h^T per 128-key chunk),
softmax without max-subtraction (scores bounded), causal mask via
affine_select (GPSIMD) on diagonal blocks only; chunks strictly above the
diagonal are skipped.  The g=0/g=1 batch streams are interleaved
instruction-by-instruction: exp on the Activation engine is the pacing stage
(~80us total), and the PE always has the other stream's matmuls to run,
which both fills its gaps and keeps its p-state at the full 2.4GHz clock
(idle gaps drop it to 1.2GHz and every matmul gets ~2x slower).
PV stationary is [128, 128] per (kc, head-slot): col 0 = ones (sum of exp
lands in PSUM row 0, base-0 aligned so reciprocal_approx_fast can read it
in place -- custom DVE ops silently corrupt on non-aligned partition bases),
cols 64:128 = V (O lands in rows 64:128; partition accesses must start at a
multiple of 32, and >32-partition accesses at 0 or 64).  The recip is
broadcast on GPSIMD (partition_broadcast) and one DVE multiply reading the
PSUM accumulator directly produces the routed output -- no PE broadcast, no
extra PSUM, so all 4 live PV accumulators fit the shared pool.

PSUM budget (8 banks): ps_s 2 x [128,2,512] pair tiles (4 banks) double-
buffer the score->exp handoff; pps 4 x 1-bank slots round-robin projection
groups, the 4 live PV accumulators, and FFN groups.
All transposes are TensorE identity-matmul transposes.  LayerNorm scales
are folded into the weights on the host; b2/ones tiles are bf16 (an fp32
operand puts the PE in 4x-slower fp32 mode).
"""

import numpy as np
import ml_dtypes

B, S, H = 2, 2048, 1024
HEADS, D = 16, 64
FFN = 4096
EPS = 1e-5
NCORES = 8
TOK = 512          # tokens per core
TT = TOK // 128    # token tiles per core (4)
FH = H // 128      # 128-feature chunks of hidden (8)
FC = FFN // 128    # 128-feature chunks of ffn (32)
KCH = S // 128     # 128-key chunks per batch (16)
QB = S // TOK      # query blocks (4)

_cache = {}
DEBUG = False


def _body(tc):
    from contextlib import ExitStack
    from concourse import mybir
    nc = tc.nc
    f32 = mybir.dt.float32
    bf16 = mybir.dt.bfloat16
    AF = mybir.ActivationFunctionType
    ALU = mybir.AluOpType

    x_d = nc.dram_tensor("x", [TOK, H], f32, kind="ExternalInput")
    wq_d = nc.dram_tensor("wq", [H, H], bf16, kind="ExternalInput")
    wk_d = nc.dram_tensor("wk", [H, H], bf16, kind="ExternalInput")
    wv_d = nc.dram_tensor("wv", [H, H], bf16, kind="ExternalInput")
    wo_d = nc.dram_tensor("wo", [H, H], bf16, kind="ExternalInput")
    w1_d = nc.dram_tensor("w1", [H, FFN], bf16, kind="ExternalInput")
    w2_d = nc.dram_tensor("w2", [FFN, H], bf16, kind="ExternalInput")
    b1_d = nc.dram_tensor("b1", [FFN, 1], f32, kind="ExternalInput")
    b2_d = nc.dram_tensor("b2", [1, H], bf16, kind="ExternalInput")
    eye_d = nc.dram_tensor("eye", [128, 128], bf16, kind="ExternalInput")
    out_d = nc.dram_tensor("out", [TOK, H], f32, kind="ExternalOutput")

    # A2A(V): chunk j = token-major V feature cols [j*128,(j+1)*128) (= head
    # pair {2j,2j+1}) of my 512 tokens; rows = token%128, col blocks = token
    # tile t.  A2A(KQ): chunk j rows = [K^T f=j (128) | Q^T f=j (128)].
    ccv_in = nc.dram_tensor("ccv_in", [8 * 128, TOK], bf16)
    ccv_out = nc.dram_tensor("ccv_out", [8 * 128, TOK], bf16)
    cc1_in = nc.dram_tensor("cc1_in", [8 * 256, TOK], bf16)
    cc1_out = nc.dram_tensor("cc1_out", [8 * 256, TOK], bf16)
    # A2A #2 (8 ranks): shard d rows [d*128,(d+1)*128) = O^T of my 2 heads for
    # dest core d's tokens (d = batch*4 + qb); split into token halves a/b so
    # o_proj of tokens 0:256 overlaps the second collective.
    cc2_in = [nc.dram_tensor(f"cc2_{i}_in", [8 * 128, 256], bf16)
              for i in range(2)]
    cc2_out = [nc.dram_tensor(f"cc2_{i}_out", [8 * 128, 256], bf16)
               for i in range(2)]
    ccw_in = nc.dram_tensor("ccw_in", [8, 64], bf16)
    ccw_out = nc.dram_tensor("ccw_out", [8, 64], bf16)
    if DEBUG:
        dbg_q = nc.dram_tensor("dbg_q", [128, S], bf16, kind="ExternalOutput")
        dbg_k = nc.dram_tensor("dbg_k", [128, S], bf16, kind="ExternalOutput")
        dbg_v = nc.dram_tensor("dbg_v", [128, KCH * 256], bf16, kind="ExternalOutput")
        dbg_ot = nc.dram_tensor("dbg_ot", [64, TOK], bf16, kind="ExternalOutput")
        dbg_o65 = nc.dram_tensor("dbg_o65", [65, TOK], f32, kind="ExternalOutput")
        dbg_rb = nc.dram_tensor("dbg_rb", [64, TOK], f32, kind="ExternalOutput")
        dbg_rcp = nc.dram_tensor("dbg_rcp", [1, TOK], f32, kind="ExternalOutput")
        dbg_pt = nc.dram_tensor("dbg_pt", [128, 2, TOK], bf16, kind="ExternalOutput")
        dbg_h2 = nc.dram_tensor("dbg_h2", [128, H], f32, kind="ExternalOutput")

    with ExitStack() as ctx:
        def pool(name, bufs, space="SBUF"):
            return ctx.enter_context(tc.tile_pool(name=name, bufs=bufs, space=space))

        pw = pool("pw", 16)      # qkvo weights, then w2 in groups (tag "w")
        pw1 = pool("pw1", 8)     # w1 column-quarters [128, 1024]
        px = pool("px", 4)       # x f32
        ph2 = pool("ph2", 4)     # post-attention residual f32
        py = pool("py", 4)       # FFN f32 accumulator
        pxn = pool("pxn", 2)     # LN outputs token-major bf16
        pxnT = pool("pxnT", 1)   # feature-major LN outputs (one [128,FH,TOK] tile)
        pqkT = pool("pqkT", 2)   # local Q^T / K^T staging
        pvl = pool("pvl", 1)     # local V staging
        pah = pool("pah", 2)     # per-head gathered qTh/kTh/v65
        pp = pool("pp", 16)      # P = exp(S) tiles
        pno = pool("pno", 2)     # O+sum [65,512] f32 drains + recip rows
        pg = pool("pg", 8)       # GELU outputs
        psm = pool("psm", 4)     # small stats
        pb = pool("pb", 1)       # biases
        pones = pool("pones", 1)
        ps_s = pool("ps_s", 2, "PSUM")   # score pairs [128, 2, TOK] (2 banks each)
        pps = pool("pps", 4, "PSUM")     # shared 1-bank slots: proj groups,
                                         # PV accumulators, recip broadcasts

        dma = nc.sync.dma_start

        # tiny warmup A2A issued before anything else: the collective-network
        # init (~60us) is lazy -- it runs at the first collective trigger --
        # so trigger it at t~0 and let it overlap LN1 + V-proj
        dma(ccw_in[:], eye_d[0:8, 0:64])
        nc.gpsimd.collective_compute(
            "AllToAll", ALU.bypass, replica_groups=[list(range(8))],
            ins=[ccw_in[:]], outs=[ccw_out[:]],
        )

        ones_bf = pones.tile([1, 128], bf16, tag="ones")
        nc.vector.memset(ones_bf[:], 1.0)
        eye_sb = pones.tile([128, 128], bf16, tag="eye")
        dma(eye_sb[:], eye_d[:])
        eps_sb = pones.tile([128, 1], f32, tag="eps")
        nc.vector.memset(eps_sb[:], EPS)
        b2_sb = pb.tile([1, 1024], bf16, tag="b2")
        dma(b2_sb[:], b2_d[:])
        b1_sb = pb.tile([128, FC], f32, tag="b1")
        dma(b1_sb[:], b1_d.rearrange("(fc p) one -> p (fc one)", p=128))

        # ---------------- weights & inputs ----------------
        # allocation order matters: slots recycle in allocation order, so
        # allocate in the order weights are *released* (V, K, Q proj, then o)
        wv_t = [pw.tile([128, 1024], bf16, tag="w", name=f"wv{i}") for i in range(FH)]
        wk_t = [pw.tile([128, 1024], bf16, tag="w", name=f"wk{i}") for i in range(FH)]
        wq_t = [pw.tile([128, 1024], bf16, tag="w", name=f"wq{i}") for i in range(FH)]
        wo_t = [pw.tile([128, 1024], bf16, tag="w", name=f"wo{i}") for i in range(FH)]
        x_t = [px.tile([128, 1024], f32, tag="x", name=f"x{t}") for t in range(TT)]
        for t in range(TT):
            dma(x_t[t][:, 0:512], x_d[t * 128:(t + 1) * 128, 0:512])
            dma(x_t[t][:, 512:1024], x_d[t * 128:(t + 1) * 128, 512:1024])
        for i in range(FH):
            dma(wv_t[i][:], wv_d[i * 128:(i + 1) * 128, :])
            dma(wk_t[i][:], wk_d[i * 128:(i + 1) * 128, :])
        for i in range(FH):
            dma(wq_t[i][:], wq_d[i * 128:(i + 1) * 128, :])
        for i in range(FH):
            dma(wo_t[i][:], wo_d[i * 128:(i + 1) * 128, :])
        # FFN colq-0 W1 issued up front: its transfers land in the early
        # stream instead of colliding with the A2A#2 wires
        w1_0 = [pw1.tile([128, 1024], bf16, tag="w1", name=f"w1_0_{i}")
                for i in range(FH)]
        for k in range(FH):
            dma(w1_0[k][:], w1_d[k * 128:(k + 1) * 128, 0:1024])

        # ---------------- LN (token-major) ----------------
        def layer_norm(x_tiles):
            xn = []
            for t in range(TT):
                st = psm.tile([128, 2, 6], f32, tag="bnst", name=f"bnst{t}")
                nc.vector.bn_stats(st[:, 0, :], x_tiles[t][:, 0:512])
                nc.vector.bn_stats(st[:, 1, :], x_tiles[t][:, 512:1024])
                mv = psm.tile([128, 2], f32, tag="bnmv", name=f"bnmv{t}")
                nc.vector.bn_aggr(mv[:], st[:])
                std = psm.tile([128, 1], f32, tag="bnsd", name=f"bnsd{t}")
                nc.scalar.activation(std[:], mv[:, 1:2], AF.Sqrt, bias=eps_sb[:])
                rstd = psm.tile([128, 1], f32, tag="bnrs", name=f"bnrs{t}")
                nc.vector.reciprocal(rstd[:], std[:])
                xo = pxn.tile([128, 1024], bf16, tag="xn", name=f"xn_{t}")
                nc.vector.tensor_scalar(xo[:], x_tiles[t][:],
                                        mv[:, 0:1], rstd[:],
                                        op0=ALU.subtract, op1=ALU.mult)
                xn.append(xo)
            return xn

        def transpose_tm_to_fm(xn, tag, nm):
            """[TT x (128tok, 1024f)] -> [128, FH, 512tok] via TensorE
            transpose (identity matmul) + DVE PSUM->SBUF copy."""
            xT = pxnT.tile([128, FH, TOK], bf16, tag=tag, name=nm)
            for t in range(TT):
                for f in range(FH):
                    tp = pps.tile([128, 128], bf16, tag="proj", name="tp")
                    nc.tensor.transpose(tp[:], xn[t][:, f * 128:(f + 1) * 128],
                                        eye_sb[:])
                    nc.vector.tensor_copy(xT[:, f, t * 128:(t + 1) * 128], tp[:])
            return xT

        xn1 = layer_norm(x_t)
        xn1T = transpose_tm_to_fm(xn1, "xnT", "xn1T")

        # ---------------- K/Q/V projections + A2As ----------------
        # V token-major [TT x (128tok, 1024f)] + its A2A -- V goes FIRST:
        # its collective absorbs the init/skew window, and V is fully
        # delivered before attention starts, so the (ACT-pacing) exp
        # stream never jams on PV waiting for V.  KQ's mesh then starts
        # with all ranks already waiting (zero input-skew wait)., split into token
        # halves so the first half's wire starts as soon as t=0,1 project
        for t in range(TT):
            vrow = pvl.tile([128, 1024], bf16, tag="vl", name=f"vl{t}")
            for h2 in range(2):
                ps = pps.tile([128, TOK], f32, tag="proj", name="ps_proj")
                for k in range(FH):
                    nc.tensor.matmul(ps[:], xn1T[:, k, t * 128:(t + 1) * 128],
                                     wv_t[k][:, h2 * 512:(h2 + 1) * 512],
                                     start=(k == 0), stop=(k == FH - 1))
                nc.vector.tensor_copy(vrow[:, h2 * 512:(h2 + 1) * 512], ps[:])
            nc.scalar.dma_start(
                ccv_in.rearrange("(j p) w -> p j w", p=128)[
                    :, :, t * 128:(t + 1) * 128],
                vrow[:].rearrange("p (j c) -> p j c", j=8))
        nc.gpsimd.collective_compute(
            "AllToAll", ALU.bypass, replica_groups=[list(range(8))],
            ins=[ccv_in[:]], outs=[ccv_out[:]],
        )

        # K^T, Q^T feature-major [FH x (128, 512)], one combined A2A fired
        # as soon as both projections are packed: its trigger (~58us) beats
        # the warmup-init completion (~74us), so all ranks are already
        # waiting and the wire starts immediately.  V (token-major) projects
        # during the KQ wire; its A2A rides right behind.  Coarse V pack:
        # chunk j gets plain column slices [128 tok, 128 feat].
        def proj_fm(w_t, tag):
            outT = []
            for f in range(FH):
                ps = pps.tile([128, TOK], f32, tag="proj", name="ps_proj")
                for k in range(FH):
                    nc.tensor.matmul(ps[:], w_t[k][:, f * 128:(f + 1) * 128],
                                     xn1T[:, k, :], start=(k == 0),
                                     stop=(k == FH - 1))
                o = pqkT.tile([128, TOK], bf16, tag=tag, name=f"{tag}_{f}")
                nc.vector.tensor_copy(o[:], ps[:])
                outT.append(o)
            return outT

        kT_l = proj_fm(wk_t, "kTl")
        for f in range(FH):
            nc.scalar.dma_start(cc1_in[f * 256:f * 256 + 128, :], kT_l[f][:])
        qT_l = proj_fm(wq_t, "qTl")
        for f in range(FH):
            nc.scalar.dma_start(cc1_in[f * 256 + 128:(f + 1) * 256, :],
                                qT_l[f][:])
        nc.gpsimd.collective_compute(
            "AllToAll", ALU.bypass, replica_groups=[list(range(8))],
            ins=[cc1_in[:]], outs=[cc1_out[:]],
        )

        # ------- attention: both batches' pair streams interleaved ---------
        # Q^T/K^T head-pair tiles [128, 2048]: partitions 0:64 = head slot 0,
        # 64:128 = slot 1.  The two S matmuls (contract=64 each) write the two
        # halves of one [128, 2, TOK] PSUM pair tile; exp and the causal
        # affine_select run once per pair.  The g=0/g=1 streams alternate
        # instruction-by-instruction: exp (ACT engine) is the pacing stage, so
        # the PE always has the other stream's matmuls to run -- no idle gaps,
        # which also keeps the PE p-state at full clock.
        qThs, kThs, vbs = [], [], []
        for g in range(2):
            qTh = pah.tile([128, S], bf16, tag="qTh", name=f"qTh{g}")
            kTh = pah.tile([128, S], bf16, tag="kTh", name=f"kTh{g}")
            kq = cc1_out.rearrange("(r c p) w -> p c r w", c=2, p=128)
            dma(kTh[:].rearrange("p (i w) -> p i w", i=4),
                kq[:, 0, 4 * g:4 * g + 4, :])
            dma(qTh[:].rearrange("p (i w) -> p i w", i=4),
                kq[:, 1, 4 * g:4 * g + 4, :])
            # PV stationary per (kc, hh) = vb[:, kc, hh*128 : hh*128+128]:
            # col 0 = ones (sum of exp -> PSUM row 0, base-0 aligned for
            # reciprocal_approx_fast), cols 64:128 = V (O -> PSUM rows
            # 64:128; partition accesses must start at 0 or 64).  Cols
            # 1:64 stay uninitialized -- they only feed PSUM rows 1:64,
            # which are never read.
            vb = pah.tile([128, KCH, 256], bf16, tag="vb", name=f"vb{g}")
            nc.vector.memset(vb[:, :, 0:1], 1.0)
            nc.vector.memset(vb[:, :, 128:129], 1.0)
            vv = ccv_out.rearrange("(r p) (t c) -> p r t c", p=128, t=4)
            for hh in range(2):
                for i in range(4):
                    dma(vb[:, i * 4:(i + 1) * 4,
                           hh * 128 + 64:hh * 128 + 128],
                        vv[:, 4 * g + i, :, hh * 64:(hh + 1) * 64])
            qThs.append(qTh); kThs.append(kTh); vbs.append(vb)
        # FFN colq-0 W2 issued here (not earlier): its pw slots are held by
        # wv until V-proj ends (~88us) -- issuing before the attention
        # section would let its 2MB transfer fire right on top of the KQ
        # wire; from here its SP configs process after the gather configs,
        # landing the transfer in the quiet attention window instead
        w2_0 = [pw.tile([128, 1024], bf16, tag="w", name=f"w2_0_{i}")
                for i in range(8)]
        for i in range(8):
            dma(w2_0[i][:], w2_d[i * 128:(i + 1) * 128, :])
        if DEBUG:
            dma(dbg_q[:], qThs[0][:])
            dma(dbg_k[:], kThs[0][:])
            dma(dbg_v.rearrange("p (kc d) -> p kc d", kc=KCH), vbs[0][:])

        def s_pair(g, qb, kc, qoff, n):
            sp = ps_s.tile([128, 2, TOK], f32, tag="sps", name="sp")
            for hh in range(2):
                nc.tensor.matmul(
                    sp[:, hh, 0:n],
                    kThs[g][64 * hh:64 * hh + 64, kc * 128:(kc + 1) * 128],
                    qThs[g][64 * hh:64 * hh + 64,
                            qb * TOK + qoff: (qb + 1) * TOK],
                    start=True, stop=True)
            pt = pp.tile([128, 2, TOK], bf16, tag="p", name="pt")
            nc.scalar.activation(pt[:, :, 0:n], sp[:, :, 0:n], AF.Exp,
                                 scale=0.125)
            return pt

        for qb in range(QB):
            o_ps = {(g, hh): pps.tile([128, TOK], f32, tag="proj",
                                      name=f"o_ps{g}{hh}")
                    for g in range(2) for hh in range(2)}
            n_pv = 4 * qb + 4

            def emit_pv(pend):
                step, kc, qoff, n, pts = pend
                for g in range(2):
                    for hh in range(2):
                        nc.tensor.matmul(
                            o_ps[(g, hh)][:, qoff:TOK],
                            vbs[g][:, kc, hh * 128:(hh + 1) * 128],
                            pts[g][:, hh, 0:n],
                            start=(step == 0), stop=(step == n_pv - 1))

            # software pipeline: each step's PV is emitted AFTER the next
            # step's score matmuls, so in the in-order PE queue the scores
            # that feed the (ACT-pacing) exp stream are never stuck behind
            # a PV burst -- exp runs gapless, PE slack absorbs PV
            pending = None
            for step in range(n_pv):
                kc = step
                if step < 4 * qb:
                    qoff, n = 0, TOK
                    pts = [s_pair(g, qb, kc, qoff, n) for g in range(2)]
                else:
                    lc = step - 4 * qb
                    qoff, n = lc * 128, TOK - (step - 4 * qb) * 128
                    pts = []
                    for g in range(2):
                        pt = s_pair(g, qb, kc, qoff, n)
                        # keep where query - key >= 0, same for both halves
                        nc.gpsimd.affine_select(
                            pt[:, :, 0:n], pt[:, :, 0:n],
                            pattern=[[0, 2], [128, 4 - lc], [1, 128]],
                            compare_op=ALU.is_ge, fill=0.0,
                            base=0, channel_multiplier=-1)
                        pts.append(pt)
                    if DEBUG and qb == 0 and lc == 0:
                        dma(dbg_pt[:], pts[0][:])
                if pending is not None:
                    emit_pv(pending)
                pending = (step, kc, qoff, n, pts)
            emit_pv(pending)

            # stage-widened normalize: all 4 reciprocals, then all 4
            # GPSIMD broadcasts, then all 4 multiplies -- the DVE and GPS
            # stages overlap across the (g, hh) chains instead of
            # ping-ponging, shortening the tail before the A2A#2 trigger.
            # PSUM row 0 = sum of exp; reciprocal reads it straight from
            # PSUM; the broadcast fills a full 128-row tile so the hh1
            # multiply stays partition-base-aligned (all operands at 64).
            recips, rbs, ots = {}, {}, {}
            for g in range(2):
                ots[g] = pno.tile([128, TOK], bf16, tag="ot", name="ot")
                for hh in range(2):
                    recips[(g, hh)] = pno.tile([1, TOK], f32, tag="recip",
                                               name="recip")
                    nc.vector.reciprocal_approx_fast(
                        recips[(g, hh)][:], o_ps[(g, hh)][0:1, :])
            for g in range(2):
                for hh in range(2):
                    rbs[(g, hh)] = pno.tile([128, TOK], f32, tag="rb",
                                            name="rb")
                    nc.gpsimd.partition_broadcast(rbs[(g, hh)][:],
                                                  recips[(g, hh)][:])
            if DEBUG and qb == 0:
                dma(dbg_rcp[:], recips[(0, 0)][:])
            for g in range(2):
                for hh in range(2):
                    nc.vector.tensor_tensor(
                        ots[g][hh * 64:(hh + 1) * 64, :],
                        o_ps[(g, hh)][64:128, :],
                        rbs[(g, hh)][hh * 64:(hh + 1) * 64, :], op=ALU.mult)
            for g in range(2):
                d = g * 4 + qb
                if DEBUG and g == 0 and qb == 0:
                    dma(dbg_ot[:], ots[g][0:64, :])
                for u in range(2):
                    dma(cc2_in[u][d * 128:(d + 1) * 128, :],
                        ots[g][:, u * 256:(u + 1) * 256])

        # ---------------- A2A #2 (token halves) ----------------
        for u in range(2):
            nc.gpsimd.collective_compute(
                "AllToAll", ALU.bypass, replica_groups=[list(range(8))],
                ins=[cc2_in[u][:]], outs=[cc2_out[u][:]],
            )

        # O^T feature-major per token half (rows = features)
        pOT = pool("pOT", 2)
        oT_q = []
        for u in range(2):
            oT = pOT.tile([128, FH, 256], bf16, tag="oT", name=f"oT{u}")
            dma(oT[:], cc2_out[u].rearrange("(f p) w -> p f w", f=FH))
            oT_q.append(oT)

        # ---------------- o_proj + residual ----------------
        h2_t = []
        for t in range(TT):
            hrow = ph2.tile([128, 1024], f32, tag="h2", name=f"h2_{t}")
            for half in range(2):
                ps = pps.tile([128, TOK], f32, tag="proj", name="ps_proj")
                oT = oT_q[t // 2]
                tc_off = (t % 2) * 128
                for k in range(FH):
                    nc.tensor.matmul(ps[:], oT[:, k, tc_off:tc_off + 128],
                                     wo_t[k][:, half * 512:(half + 1) * 512],
                                     start=(k == 0), stop=(k == FH - 1))
                nc.vector.tensor_tensor(hrow[:, half * 512:(half + 1) * 512],
                                        ps[:], x_t[t][:, half * 512:(half + 1) * 512],
                                        op=ALU.add)
            h2_t.append(hrow)
        if DEBUG:
            dma(dbg_h2[:], h2_t[0][:])

        # ---------------- LN2 + FFN ----------------
        xn2 = layer_norm(h2_t)
        xn2T = transpose_tm_to_fm(xn2, "xnT", "xn2T")

        # W2 tiles reuse the "w" slots as q/k/v weights release.
        # FFN runs in 4 column-quarter passes: W1 quarter -> GELU -> W2 group
        # partial-accumulated into an SBUF f32 accumulator.
        y_sb = [py.tile([128, 1024], f32, tag="y", name=f"y{t}") for t in range(TT)]
        for colq in range(4):
            if colq == 0:
                w1_t, w2_t = w1_0, w2_0
            elif True:
                w1_t = [pw1.tile([128, 1024], bf16, tag="w1",
                                 name=f"w1_{colq}_{i}") for i in range(FH)]
                for k in range(FH):
                    dma(w1_t[k][:], w1_d[k * 128:(k + 1) * 128,
                                         colq * 1024:(colq + 1) * 1024])
                w2_t = [pw.tile([128, 1024], bf16, tag="w",
                                name=f"w2_{colq}_{i}") for i in range(8)]
                for i in range(8):
                    dma(w2_t[i][:], w2_d[(colq * 8 + i) * 128:
                                         (colq * 8 + i + 1) * 128, :])
            g_t = []
            for fc_l in range(8):
                fc = colq * 8 + fc_l
                ps = pps.tile([128, TOK], f32, tag="proj", name="ps_proj")
                for k in range(FH):
                    nc.tensor.matmul(ps[:],
                                     w1_t[k][:, fc_l * 128:(fc_l + 1) * 128],
                                     xn2T[:, k, :], start=(k == 0),
                                     stop=(k == FH - 1))
                g = pg.tile([128, TOK], bf16, tag="g", name=f"g{fc}")
                nc.scalar.activation(g[:], ps[:], AF.Gelu,
                                     bias=b1_sb[:, fc:fc + 1])
                g_t.append(g)
            for t in range(TT):
                for half in range(2):
                    ps = pps.tile([128, TOK], f32, tag="proj", name="ps_proj")
                    for i in range(8):
                        nc.tensor.matmul(ps[:], g_t[i][:, t * 128:(t + 1) * 128],
                                         w2_t[i][:, half * 512:(half + 1) * 512],
                                         start=(i == 0), stop=(i == 7 and colq != 0))
                    if colq == 0:
                        # fold b2 + attention residual h2 into the accumulator
                        nc.tensor.matmul(ps[:], ones_bf[:, 0:128],
                                         b2_sb[:, half * 512:(half + 1) * 512],
                                         start=False, stop=True)
                        nc.vector.tensor_tensor(
                            y_sb[t][:, half * 512:(half + 1) * 512], ps[:],
                            h2_t[t][:, half * 512:(half + 1) * 512], op=ALU.add)
                    else:
                        nc.vector.tensor_tensor(
                            y_sb[t][:, half * 512:(half + 1) * 512], ps[:],
                            y_sb[t][:, half * 512:(half + 1) * 512], op=ALU.add)
        for t in range(TT):
            dma(out_d[t * 128:(t + 1) * 128, :], y_sb[t][:])

def _build():
    from concourse import bacc, tile
    nc = bacc.Bacc("TRN2", target_bir_lowering=False, debug=False,
                   num_devices=NCORES)
    with tile.TileContext(nc) as tc:
        _body(tc)
    nc.compile()
    return nc


def _prepare_in_maps(inputs):
    bf = ml_dtypes.bfloat16
    x = np.asarray(inputs["hidden_states"], np.float32)
    ln1s = np.asarray(inputs["ln1_scale"], np.float32)
    ln2s = np.asarray(inputs["ln2_scale"], np.float32)
    # LN scale folding (shifts are zero in this problem)
    wq = (ln1s[:, None] * np.asarray(inputs["Wq"], np.float32)).astype(bf)
    wk = (ln1s[:, None] * np.asarray(inputs["Wk"], np.float32)).astype(bf)
    wv = (ln1s[:, None] * np.asarray(inputs["Wv"], np.float32)).astype(bf)
    wo = np.asarray(inputs["Wo"], np.float32).astype(bf)
    w1 = (ln2s[:, None] * np.asarray(inputs["W1"], np.float32)).astype(bf)
    w2 = np.asarray(inputs["W2"], np.float32).astype(bf)
    b1 = np.asarray(inputs["b1"], np.float32).reshape(FFN, 1)
    b2 = np.asarray(inputs["b2"], np.float32).reshape(1, H)
    shift2 = np.asarray(inputs["ln2_shift"], np.float32)
    # exact shift folding for the FFN branch: b1 += ln2_shift @ W1
    b1 = b1 + (shift2 @ np.asarray(inputs["W1"], np.float32)).reshape(FFN, 1)
    b2 = b2.astype(bf)
    in_maps = []
    for c in range(NCORES):
        g, r = divmod(c, 4)
        in_maps.append({
            "x": np.ascontiguousarray(x[g, r * TOK:(r + 1) * TOK, :]),
            "wq": wq, "wk": wk, "wv": wv, "wo": wo,
            "w1": w1, "w2": w2, "b1": b1, "b2": b2,
            "eye": np.eye(128, dtype=bf),
        })
    return in_maps


def kernel(**inputs):
    if "nc" not in _cache:
        _cache["nc"] = _build()
    from concourse.bass_utils import run_bass_kernel_spmd
    in_maps = _prepare_in_maps(inputs)
    res = run_bass_kernel_spmd(_cache["nc"], in_maps, core_ids=list(range(NCORES)))
    out = np.empty((B, S, H), np.float32)
    for c in range(NCORES):
        g, r = divmod(c, 4)
        out[g, r * TOK:(r + 1) * TOK, :] = res.results[c]["out"]
    return out


# revision 59
# speedup vs baseline: 2.0527x; 2.0527x over previous
"""Trainium2 distributed kernel for a transformer decoder layer (8 NeuronCores).

Layer: x -> LN1 -> causal MHA (16 heads, d=64) -> +res -> LN2 -> FFN(GELU) -> +res
hidden_states [2, 2048, 1024], HID=1024, FFN=4096, f32 I/O, bf16 matmul compute.

Sharding: 2 batch groups x 4 cores. Core c: batch g=c//4, rank r=c%4, owns
token rows [r*512,(r+1)*512): LN1, QKV projections, o_proj, LN2, FFN, output.
Attention is head-sharded across all 8 cores via AllToAll (uniform SPMD
program: every core runs full-sequence causal attention for compile-time-fixed
head slots {2c, 2c+1} x both batches; the A2A routes each core its heads).

Collective plan (the CC engine serializes collectives, and the one-time
collective-network init lands 25-90us into the run -- lazily, at the first
trigger):
  warmup A2A (tiny, first instruction) starts the init at t~0 so it overlaps
  LN1 + the projections instead of a real wire.
  A2A(K+Q): one combined collective (chunk j = [K^T f=j | Q^T f=j] of my 512
  tokens, f = head-pair) fired right after the K/Q projections.
  A2A(V): token-major V in one collective right behind KQ (a token-half
  split measured even: the extra mesh cost what the earlier first half
  saved -- the software-pipelined PV backlog absorbs V's arrival anyway).
  A2A #2a/2b: normalized attention output O^T routed back token-major, in
  token halves so o_proj of tokens 0:256 overlaps #2b.
DMA discipline: every dma_start costs ~0.6us of CONFIG time on its issuing
engine's sequencer, so scatter/gathers use single multi-dim-AP DMAs (max 3
dims after the partition dim).  GPSIMD-issued DMAs are NOT a shortcut: the
cost model's 25ns config figure is wrong on this stack -- measured runs
regressed ~90us (software descriptor generation on the DSP); keep bulk DMAs
on nc.sync/nc.scalar.

Attention: scores keys-on-partitions (S^T = K_h @ Q_h^T per 128-key chunk),
softmax without max-subtraction (scores bounded), causal mask via
affine_select (GPSIMD) on diagonal blocks only; chunks strictly above the
diagonal are skipped.  The g=0/g=1 batch streams are interleaved
instruction-by-instruction: exp on the Activation engine is the pacing stage
(~80us total), and the PE always has the other stream's matmuls to run,
which both fills its gaps and keeps its p-state at the full 2.4GHz clock
(idle gaps drop it to 1.2GHz and every matmul gets ~2x slower).
PV stationary is [128, 128] per (kc, head-slot): col 0 = ones (sum of exp
lands in PSUM row 0, base-0 aligned so reciprocal_approx_fast can read it
in place -- custom DVE ops silently corrupt on non-aligned partition bases),
cols 64:128 = V (O lands in rows 64:128; partition accesses must start at a
multiple of 32, and >32-partition accesses at 0 or 64).  The recip is
broadcast on GPSIMD (partition_broadcast) and one DVE multiply reading the
PSUM accumulator directly produces the routed output -- no PE broadcast, no
extra PSUM, so all 4 live PV accumulators fit the shared pool.

PSUM budget (8 banks): ps_s 2 x [128,2,512] pair tiles (4 banks) double-
buffer the score->exp handoff; pps 4 x 1-bank slots round-robin projection
groups, the 4 live PV accumulators, and FFN groups.
All transposes are TensorE identity-matmul transposes.  LayerNorm scales
are folded into the weights on the host; b2/ones tiles are bf16 (an fp32
operand puts the PE in 4x-slower fp32 mode).
"""

import numpy as np
import ml_dtypes

B, S, H = 2, 2048, 1024
HEADS, D = 16, 64
FFN = 4096
EPS = 1e-5
NCORES = 8
TOK = 512          # tokens per core
TT = TOK // 128    # token tiles per core (4)
FH = H // 128      # 128-feature chunks of hidden (8)
FC = FFN // 128    # 128-feature chunks of ffn (32)
KCH = S // 128     # 128-key chunks per batch (16)
QB = S // TOK      # query blocks (4)

_cache = {}
DEBUG = False


def _body(tc):
    from contextlib import ExitStack
    from concourse import mybir
    nc = tc.nc
    f32 = mybir.dt.float32
    bf16 = mybir.dt.bfloat16
    AF = mybir.ActivationFunctionType
    ALU = mybir.AluOpType

    x_d = nc.dram_tensor("x", [TOK, H], f32, kind="ExternalInput")
    wq_d = nc.dram_tensor("wq", [H, H], bf16, kind="ExternalInput")
    wk_d = nc.dram_tensor("wk", [H, H], bf16, kind="ExternalInput")
    wv_d = nc.dram_tensor("wv", [H, H], bf16, kind="ExternalInput")
    wo_d = nc.dram_tensor("wo", [H, H], bf16, kind="ExternalInput")
    w1_d = nc.dram_tensor("w1", [H, FFN], bf16, kind="ExternalInput")
    w2_d = nc.dram_tensor("w2", [FFN, H], bf16, kind="ExternalInput")
    b1_d = nc.dram_tensor("b1", [FFN, 1], f32, kind="ExternalInput")
    b2_d = nc.dram_tensor("b2", [1, H], bf16, kind="ExternalInput")
    eye_d = nc.dram_tensor("eye", [128, 128], bf16, kind="ExternalInput")
    out_d = nc.dram_tensor("out", [TOK, H], f32, kind="ExternalOutput")

    # A2A(V): chunk j = token-major V feature cols [j*128,(j+1)*128) (= head
    # pair {2j,2j+1}) of my 512 tokens; rows = token%128, col blocks = token
    # tile t.  A2A(KQ): chunk j rows = [K^T f=j (128) | Q^T f=j (128)].
    ccv_in = nc.dram_tensor("ccv_in", [8 * 128, TOK], bf16)
    ccv_out = nc.dram_tensor("ccv_out", [8 * 128, TOK], bf16)
    cc1_in = nc.dram_tensor("cc1_in", [8 * 256, TOK], bf16)
    cc1_out = nc.dram_tensor("cc1_out", [8 * 256, TOK], bf16)
    # A2A #2 (8 ranks): shard d rows [d*128,(d+1)*128) = O^T of my 2 heads for
    # dest core d's tokens (d = batch*4 + qb); split into token halves a/b so
    # o_proj of tokens 0:256 overlaps the second collective.
    cc2_in = [nc.dram_tensor(f"cc2_{i}_in", [8 * 128, 256], bf16)
              for i in range(2)]
    cc2_out = [nc.dram_tensor(f"cc2_{i}_out", [8 * 128, 256], bf16)
               for i in range(2)]
    ccw_in = nc.dram_tensor("ccw_in", [8, 64], bf16)
    ccw_out = nc.dram_tensor("ccw_out", [8, 64], bf16)
    if DEBUG:
        dbg_q = nc.dram_tensor("dbg_q", [128, S], bf16, kind="ExternalOutput")
        dbg_k = nc.dram_tensor("dbg_k", [128, S], bf16, kind="ExternalOutput")
        dbg_v = nc.dram_tensor("dbg_v", [128, KCH * 256], bf16, kind="ExternalOutput")
        dbg_ot = nc.dram_tensor("dbg_ot", [64, TOK], bf16, kind="ExternalOutput")
        dbg_o65 = nc.dram_tensor("dbg_o65", [65, TOK], f32, kind="ExternalOutput")
        dbg_rb = nc.dram_tensor("dbg_rb", [64, TOK], f32, kind="ExternalOutput")
        dbg_rcp = nc.dram_tensor("dbg_rcp", [1, TOK], f32, kind="ExternalOutput")
        dbg_pt = nc.dram_tensor("dbg_pt", [128, 2, TOK], bf16, kind="ExternalOutput")
        dbg_h2 = nc.dram_tensor("dbg_h2", [128, H], f32, kind="ExternalOutput")

    with ExitStack() as ctx:
        def pool(name, bufs, space="SBUF"):
            return ctx.enter_context(tc.tile_pool(name=name, bufs=bufs, space=space))

        pw = pool("pw", 16)      # qkvo weights, then w2 in groups (tag "w")
        pw1 = pool("pw1", 8)     # w1 column-quarters [128, 1024]
        px = pool("px", 4)       # x f32
        ph2 = pool("ph2", 4)     # post-attention residual f32
        py = pool("py", 4)       # FFN f32 accumulator
        pxn = pool("pxn", 2)     # LN outputs token-major bf16
        pxnT = pool("pxnT", 1)   # feature-major LN outputs (one [128,FH,TOK] tile)
        pqkT = pool("pqkT", 2)   # local Q^T / K^T staging
        pvl = pool("pvl", 1)     # local V staging
        pah = pool("pah", 2)     # per-head gathered qTh/kTh/v65
        pp = pool("pp", 16)      # P = exp(S) tiles
        pno = pool("pno", 2)     # O+sum [65,512] f32 drains + recip rows
        pg = pool("pg", 8)       # GELU outputs
        psm = pool("psm", 4)     # small stats
        pb = pool("pb", 1)       # biases
        pones = pool("pones", 1)
        ps_s = pool("ps_s", 2, "PSUM")   # score pairs [128, 2, TOK] (2 banks each)
        pps = pool("pps", 4, "PSUM")     # shared 1-bank slots: proj groups,
                                         # PV accumulators, recip broadcasts

        dma = nc.sync.dma_start

        # tiny warmup A2A issued before anything else: the collective-network
        # init (~60us) is lazy -- it runs at the first collective trigger --
        # so trigger it at t~0 and let it overlap LN1 + V-proj
        dma(ccw_in[:], eye_d[0:8, 0:64])
        nc.gpsimd.collective_compute(
            "AllToAll", ALU.bypass, replica_groups=[list(range(8))],
            ins=[ccw_in[:]], outs=[ccw_out[:]],
        )

        ones_bf = pones.tile([1, 128], bf16, tag="ones")
        nc.vector.memset(ones_bf[:], 1.0)
        eye_sb = pones.tile([128, 128], bf16, tag="eye")
        dma(eye_sb[:], eye_d[:])
        eps_sb = pones.tile([128, 1], f32, tag="eps")
        nc.vector.memset(eps_sb[:], EPS)
        b2_sb = pb.tile([1, 1024], bf16, tag="b2")
        dma(b2_sb[:], b2_d[:])
        b1_sb = pb.tile([128, FC], f32, tag="b1")
        dma(b1_sb[:], b1_d.rearrange("(fc p) one -> p (fc one)", p=128))

        # ---------------- weights & inputs ----------------
        # allocation order matters: slots recycle in allocation order, so
        # allocate in the order weights are *released* (V, K, Q proj, then o)
        wk_t = [pw.tile([128, 1024], bf16, tag="w", name=f"wk{i}") for i in range(FH)]
        wq_t = [pw.tile([128, 1024], bf16, tag="w", name=f"wq{i}") for i in range(FH)]
        wv_t = [pw.tile([128, 1024], bf16, tag="w", name=f"wv{i}") for i in range(FH)]
        wo_t = [pw.tile([128, 1024], bf16, tag="w", name=f"wo{i}") for i in range(FH)]
        x_t = [px.tile([128, 1024], f32, tag="x", name=f"x{t}") for t in range(TT)]
        for t in range(TT):
            dma(x_t[t][:, 0:512], x_d[t * 128:(t + 1) * 128, 0:512])
            dma(x_t[t][:, 512:1024], x_d[t * 128:(t + 1) * 128, 512:1024])
        for i in range(FH):
            dma(wk_t[i][:], wk_d[i * 128:(i + 1) * 128, :])
            dma(wq_t[i][:], wq_d[i * 128:(i + 1) * 128, :])
        for i in range(FH):
            dma(wv_t[i][:], wv_d[i * 128:(i + 1) * 128, :])
        for i in range(FH):
            dma(wo_t[i][:], wo_d[i * 128:(i + 1) * 128, :])
        # FFN colq-0 W1 issued up front: its transfers land in the early
        # stream instead of colliding with the A2A#2 wires
        w1_0 = [pw1.tile([128, 1024], bf16, tag="w1", name=f"w1_0_{i}")
                for i in range(FH)]
        for k in range(FH):
            dma(w1_0[k][:], w1_d[k * 128:(k + 1) * 128, 0:1024])

        # ---------------- LN (token-major) ----------------
        def layer_norm(x_tiles):
            xn = []
            for t in range(TT):
                st = psm.tile([128, 2, 6], f32, tag="bnst", name=f"bnst{t}")
                nc.vector.bn_stats(st[:, 0, :], x_tiles[t][:, 0:512])
                nc.vector.bn_stats(st[:, 1, :], x_tiles[t][:, 512:1024])
                mv = psm.tile([128, 2], f32, tag="bnmv", name=f"bnmv{t}")
                nc.vector.bn_aggr(mv[:], st[:])
                std = psm.tile([128, 1], f32, tag="bnsd", name=f"bnsd{t}")
                nc.scalar.activation(std[:], mv[:, 1:2], AF.Sqrt, bias=eps_sb[:])
                rstd = psm.tile([128, 1], f32, tag="bnrs", name=f"bnrs{t}")
                nc.vector.reciprocal(rstd[:], std[:])
                xo = pxn.tile([128, 1024], bf16, tag="xn", name=f"xn_{t}")
                nc.vector.tensor_scalar(xo[:], x_tiles[t][:],
                                        mv[:, 0:1], rstd[:],
                                        op0=ALU.subtract, op1=ALU.mult)
                xn.append(xo)
            return xn

        def transpose_tm_to_fm(xn, tag, nm):
            """[TT x (128tok, 1024f)] -> [128, FH, 512tok] via TensorE
            transpose (identity matmul) + DVE PSUM->SBUF copy."""
            xT = pxnT.tile([128, FH, TOK], bf16, tag=tag, name=nm)
            for t in range(TT):
                for f in range(FH):
                    tp = pps.tile([128, 128], bf16, tag="proj", name="tp")
                    nc.tensor.transpose(tp[:], xn[t][:, f * 128:(f + 1) * 128],
                                        eye_sb[:])
                    nc.vector.tensor_copy(xT[:, f, t * 128:(t + 1) * 128], tp[:])
            return xT

        xn1 = layer_norm(x_t)
        xn1T = transpose_tm_to_fm(xn1, "xnT", "xn1T")

        # ---------------- K/Q/V projections + A2As ----------------
        # K^T, Q^T feature-major [FH x (128, 512)], one combined A2A fired
        # as soon as both projections are packed: its trigger (~58us) beats
        # the warmup-init completion (~74us), so all ranks are already
        # waiting and the wire starts immediately.  V (token-major) projects
        # during the KQ wire; its A2A rides right behind.  Coarse V pack:
        # chunk j gets plain column slices [128 tok, 128 feat].
        def proj_fm(w_t, tag):
            outT = []
            for f in range(FH):
                ps = pps.tile([128, TOK], f32, tag="proj", name="ps_proj")
                for k in range(FH):
                    nc.tensor.matmul(ps[:], w_t[k][:, f * 128:(f + 1) * 128],
                                     xn1T[:, k, :], start=(k == 0),
                                     stop=(k == FH - 1))
                o = pqkT.tile([128, TOK], bf16, tag=tag, name=f"{tag}_{f}")
                nc.vector.tensor_copy(o[:], ps[:])
                outT.append(o)
            return outT

        kT_l = proj_fm(wk_t, "kTl")
        for f in range(FH):
            nc.scalar.dma_start(cc1_in[f * 256:f * 256 + 128, :], kT_l[f][:])
        qT_l = proj_fm(wq_t, "qTl")
        for f in range(FH):
            nc.scalar.dma_start(cc1_in[f * 256 + 128:(f + 1) * 256, :],
                                qT_l[f][:])
        nc.gpsimd.collective_compute(
            "AllToAll", ALU.bypass, replica_groups=[list(range(8))],
            ins=[cc1_in[:]], outs=[cc1_out[:]],
        )

        # V token-major [TT x (128tok, 1024f)] + its A2A, split into token
        # halves so the first half's wire starts as soon as t=0,1 project
        for t in range(TT):
            vrow = pvl.tile([128, 1024], bf16, tag="vl", name=f"vl{t}")
            for h2 in range(2):
                ps = pps.tile([128, TOK], f32, tag="proj", name="ps_proj")
                for k in range(FH):
                    nc.tensor.matmul(ps[:], xn1T[:, k, t * 128:(t + 1) * 128],
                                     wv_t[k][:, h2 * 512:(h2 + 1) * 512],
                                     start=(k == 0), stop=(k == FH - 1))
                nc.vector.tensor_copy(vrow[:, h2 * 512:(h2 + 1) * 512], ps[:])
            nc.scalar.dma_start(
                ccv_in.rearrange("(j p) w -> p j w", p=128)[
                    :, :, t * 128:(t + 1) * 128],
                vrow[:].rearrange("p (j c) -> p j c", j=8))
        nc.gpsimd.collective_compute(
            "AllToAll", ALU.bypass, replica_groups=[list(range(8))],
            ins=[ccv_in[:]], outs=[ccv_out[:]],
        )

        # ------- attention: both batches' pair streams interleaved ---------
        # Q^T/K^T head-pair tiles [128, 2048]: partitions 0:64 = head slot 0,
        # 64:128 = slot 1.  The two S matmuls (contract=64 each) write the two
        # halves of one [128, 2, TOK] PSUM pair tile; exp and the causal
        # affine_select run once per pair.  The g=0/g=1 streams alternate
        # instruction-by-instruction: exp (ACT engine) is the pacing stage, so
        # the PE always has the other stream's matmuls to run -- no idle gaps,
        # which also keeps the PE p-state at full clock.
        qThs, kThs, vbs = [], [], []
        for g in range(2):
            qTh = pah.tile([128, S], bf16, tag="qTh", name=f"qTh{g}")
            kTh = pah.tile([128, S], bf16, tag="kTh", name=f"kTh{g}")
            kq = cc1_out.rearrange("(r c p) w -> p c r w", c=2, p=128)
            dma(kTh[:].rearrange("p (i w) -> p i w", i=4),
                kq[:, 0, 4 * g:4 * g + 4, :])
            dma(qTh[:].rearrange("p (i w) -> p i w", i=4),
                kq[:, 1, 4 * g:4 * g + 4, :])
            # PV stationary per (kc, hh) = vb[:, kc, hh*128 : hh*128+128]:
            # col 0 = ones (sum of exp -> PSUM row 0, base-0 aligned for
            # reciprocal_approx_fast), cols 64:128 = V (O -> PSUM rows
            # 64:128; partition accesses must start at 0 or 64).  Cols
            # 1:64 stay uninitialized -- they only feed PSUM rows 1:64,
            # which are never read.
            vb = pah.tile([128, KCH, 256], bf16, tag="vb", name=f"vb{g}")
            nc.vector.memset(vb[:, :, 0:1], 1.0)
            nc.vector.memset(vb[:, :, 128:129], 1.0)
            vv = ccv_out.rearrange("(r p) (t c) -> p r t c", p=128, t=4)
            for hh in range(2):
                for i in range(4):
                    dma(vb[:, i * 4:(i + 1) * 4,
                           hh * 128 + 64:hh * 128 + 128],
                        vv[:, 4 * g + i, :, hh * 64:(hh + 1) * 64])
            qThs.append(qTh); kThs.append(kTh); vbs.append(vb)
        # FFN colq-0 W2 issued here (not earlier): its pw slots are held by
        # wv until V-proj ends (~88us) -- issuing before the attention
        # section would let its 2MB transfer fire right on top of the KQ
        # wire; from here its SP configs process after the gather configs,
        # landing the transfer in the quiet attention window instead
        w2_0 = [pw.tile([128, 1024], bf16, tag="w", name=f"w2_0_{i}")
                for i in range(8)]
        for i in range(8):
            dma(w2_0[i][:], w2_d[i * 128:(i + 1) * 128, :])
        if DEBUG:
            dma(dbg_q[:], qThs[0][:])
            dma(dbg_k[:], kThs[0][:])
            dma(dbg_v.rearrange("p (kc d) -> p kc d", kc=KCH), vbs[0][:])

        def s_pair(g, qb, kc, qoff, n):
            sp = ps_s.tile([128, 2, TOK], f32, tag="sps", name="sp")
            for hh in range(2):
                nc.tensor.matmul(
                    sp[:, hh, 0:n],
                    kThs[g][64 * hh:64 * hh + 64, kc * 128:(kc + 1) * 128],
                    qThs[g][64 * hh:64 * hh + 64,
                            qb * TOK + qoff: (qb + 1) * TOK],
                    start=True, stop=True)
            pt = pp.tile([128, 2, TOK], bf16, tag="p", name="pt")
            nc.scalar.activation(pt[:, :, 0:n], sp[:, :, 0:n], AF.Exp,
                                 scale=0.125)
            return pt

        for qb in range(QB):
            o_ps = {(g, hh): pps.tile([128, TOK], f32, tag="proj",
                                      name=f"o_ps{g}{hh}")
                    for g in range(2) for hh in range(2)}
            n_pv = 4 * qb + 4

            def emit_pv(pend):
                step, kc, qoff, n, pts = pend
                for g in range(2):
                    for hh in range(2):
                        nc.tensor.matmul(
                            o_ps[(g, hh)][:, qoff:TOK],
                            vbs[g][:, kc, hh * 128:(hh + 1) * 128],
                            pts[g][:, hh, 0:n],
                            start=(step == 0), stop=(step == n_pv - 1))

            # software pipeline: each step's PV is emitted AFTER the next
            # step's score matmuls, so in the in-order PE queue the scores
            # that feed the (ACT-pacing) exp stream are never stuck behind
            # a PV burst -- exp runs gapless, PE slack absorbs PV
            pending = None
            for step in range(n_pv):
                kc = step
                if step < 4 * qb:
                    qoff, n = 0, TOK
                    pts = [s_pair(g, qb, kc, qoff, n) for g in range(2)]
                else:
                    lc = step - 4 * qb
                    qoff, n = lc * 128, TOK - (step - 4 * qb) * 128
                    pts = []
                    for g in range(2):
                        pt = s_pair(g, qb, kc, qoff, n)
                        # keep where query - key >= 0, same for both halves
                        nc.gpsimd.affine_select(
                            pt[:, :, 0:n], pt[:, :, 0:n],
                            pattern=[[0, 2], [128, 4 - lc], [1, 128]],
                            compare_op=ALU.is_ge, fill=0.0,
                            base=0, channel_multiplier=-1)
                        pts.append(pt)
                    if DEBUG and qb == 0 and lc == 0:
                        dma(dbg_pt[:], pts[0][:])
                if pending is not None:
                    emit_pv(pending)
                pending = (step, kc, qoff, n, pts)
            emit_pv(pending)

            # stage-widened normalize: all 4 reciprocals, then all 4
            # GPSIMD broadcasts, then all 4 multiplies -- the DVE and GPS
            # stages overlap across the (g, hh) chains instead of
            # ping-ponging, shortening the tail before the A2A#2 trigger.
            # PSUM row 0 = sum of exp; reciprocal reads it straight from
            # PSUM; the broadcast fills a full 128-row tile so the hh1
            # multiply stays partition-base-aligned (all operands at 64).
            recips, rbs, ots = {}, {}, {}
            for g in range(2):
                ots[g] = pno.tile([128, TOK], bf16, tag="ot", name="ot")
                for hh in range(2):
                    recips[(g, hh)] = pno.tile([1, TOK], f32, tag="recip",
                                               name="recip")
                    nc.vector.reciprocal_approx_fast(
                        recips[(g, hh)][:], o_ps[(g, hh)][0:1, :])
            for g in range(2):
                for hh in range(2):
                    rbs[(g, hh)] = pno.tile([128, TOK], f32, tag="rb",
                                            name="rb")
                    nc.gpsimd.partition_broadcast(rbs[(g, hh)][:],
                                                  recips[(g, hh)][:])
            if DEBUG and qb == 0:
                dma(dbg_rcp[:], recips[(0, 0)][:])
            for g in range(2):
                for hh in range(2):
                    nc.vector.tensor_tensor(
                        ots[g][hh * 64:(hh + 1) * 64, :],
                        o_ps[(g, hh)][64:128, :],
                        rbs[(g, hh)][hh * 64:(hh + 1) * 64, :], op=ALU.mult)
            for g in range(2):
                d = g * 4 + qb
                if DEBUG and g == 0 and qb == 0:
                    dma(dbg_ot[:], ots[g][0:64, :])
                for u in range(2):
                    dma(cc2_in[u][d * 128:(d + 1) * 128, :],
                        ots[g][:, u * 256:(u + 1) * 256])

        # ---------------- A2A #2 (token halves) ----------------
        for u in range(2):
            nc.gpsimd.collective_compute(
                "AllToAll", ALU.bypass, replica_groups=[list(range(8))],
                ins=[cc2_in[u][:]], outs=[cc2_out[u][:]],
            )

        # O^T feature-major per token half (rows = features)
        pOT = pool("pOT", 2)
        oT_q = []
        for u in range(2):
            oT = pOT.tile([128, FH, 256], bf16, tag="oT", name=f"oT{u}")
            dma(oT[:], cc2_out[u].rearrange("(f p) w -> p f w", f=FH))
            oT_q.append(oT)

        # ---------------- o_proj + residual ----------------
        h2_t = []
        for t in range(TT):
            hrow = ph2.tile([128, 1024], f32, tag="h2", name=f"h2_{t}")
            for half in range(2):
                ps = pps.tile([128, TOK], f32, tag="proj", name="ps_proj")
                oT = oT_q[t // 2]
                tc_off = (t % 2) * 128
                for k in range(FH):
                    nc.tensor.matmul(ps[:], oT[:, k, tc_off:tc_off + 128],
                                     wo_t[k][:, half * 512:(half + 1) * 512],
                                     start=(k == 0), stop=(k == FH - 1))
                nc.vector.tensor_tensor(hrow[:, half * 512:(half + 1) * 512],
                                        ps[:], x_t[t][:, half * 512:(half + 1) * 512],
                                        op=ALU.add)
            h2_t.append(hrow)
        if DEBUG:
            dma(dbg_h2[:], h2_t[0][:])

        # ---------------- LN2 + FFN ----------------
        xn2 = layer_norm(h2_t)
        xn2T = transpose_tm_to_fm(xn2, "xnT", "xn2T")

        # W2 tiles reuse the "w" slots as q/k/v weights release.
        # FFN runs in 4 column-quarter passes: W1 quarter -> GELU -> W2 group
        # partial-accumulated into an SBUF f32 accumulator.
        y_sb = [py.tile([128, 1024], f32, tag="y", name=f"y{t}") for t in range(TT)]
        for colq in range(4):
            if colq == 0:
                w1_t, w2_t = w1_0, w2_0
            elif True:
                w1_t = [pw1.tile([128, 1024], bf16, tag="w1",
                                 name=f"w1_{colq}_{i}") for i in range(FH)]
                for k in range(FH):
                    dma(w1_t[k][:], w1_d[k * 128:(k + 1) * 128,
                                         colq * 1024:(colq + 1) * 1024])
                w2_t = [pw.tile([128, 1024], bf16, tag="w",
                                name=f"w2_{colq}_{i}") for i in range(8)]
                for i in range(8):
                    dma(w2_t[i][:], w2_d[(colq * 8 + i) * 128:
                                         (colq * 8 + i + 1) * 128, :])
            g_t = []
            for fc_l in range(8):
                fc = colq * 8 + fc_l
                ps = pps.tile([128, TOK], f32, tag="proj", name="ps_proj")
                for k in range(FH):
                    nc.tensor.matmul(ps[:],
                                     w1_t[k][:, fc_l * 128:(fc_l + 1) * 128],
                                     xn2T[:, k, :], start=(k == 0),
                                     stop=(k == FH - 1))
                g = pg.tile([128, TOK], bf16, tag="g", name=f"g{fc}")
                nc.scalar.activation(g[:], ps[:], AF.Gelu,
                                     bias=b1_sb[:, fc:fc + 1])
                g_t.append(g)
            for t in range(TT):
                for half in range(2):
                    ps = pps.tile([128, TOK], f32, tag="proj", name="ps_proj")
                    for i in range(8):
                        nc.tensor.matmul(ps[:], g_t[i][:, t * 128:(t + 1) * 128],
                                         w2_t[i][:, half * 512:(half + 1) * 512],
                                         start=(i == 0), stop=(i == 7 and colq != 0))
                    if colq == 0:
                        # fold b2 + attention residual h2 into the accumulator
                        nc.tensor.matmul(ps[:], ones_bf[:, 0:128],
                                         b2_sb[:, half * 512:(half + 1) * 512],
                                         start=False, stop=True)
                        nc.vector.tensor_tensor(
                            y_sb[t][:, half * 512:(half + 1) * 512], ps[:],
                            h2_t[t][:, half * 512:(half + 1) * 512], op=ALU.add)
                    else:
                        nc.vector.tensor_tensor(
                            y_sb[t][:, half * 512:(half + 1) * 512], ps[:],
                            y_sb[t][:, half * 512:(half + 1) * 512], op=ALU.add)
        for t in range(TT):
            dma(out_d[t * 128:(t + 1) * 128, :], y_sb[t][:])

def _build():
    from concourse import bacc, tile
    nc = bacc.Bacc("TRN2", target_bir_lowering=False, debug=False,
                   num_devices=NCORES)
    with tile.TileContext(nc) as tc:
        _body(tc)
    nc.compile()
    return nc


def _prepare_in_maps(inputs):
    bf = ml_dtypes.bfloat16
    x = np.asarray(inputs["hidden_states"], np.float32)
    ln1s = np.asarray(inputs["ln1_scale"], np.float32)
    ln2s = np.asarray(inputs["ln2_scale"], np.float32)
    # LN scale folding (shifts are zero in this problem)
    wq = (ln1s[:, None] * np.asarray(inputs["Wq"], np.float32)).astype(bf)
    wk = (ln1s[:, None] * np.asarray(inputs["Wk"], np.float32)).astype(bf)
    wv = (ln1s[:, None] * np.asarray(inputs["Wv"], np.float32)).astype(bf)
    wo = np.asarray(inputs["Wo"], np.float32).astype(bf)
    w1 = (ln2s[:, None] * np.asarray(inputs["W1"], np.float32)).astype(bf)
    w2 = np.asarray(inputs["W2"], np.float32).astype(bf)
    b1 = np.asarray(inputs["b1"], np.float32).reshape(FFN, 1)
    b2 = np.asarray(inputs["b2"], np.float32).reshape(1, H)
    shift2 = np.asarray(inputs["ln2_shift"], np.float32)
    # exact shift folding for the FFN branch: b1 += ln2_shift @ W1
    b1 = b1 + (shift2 @ np.asarray(inputs["W1"], np.float32)).reshape(FFN, 1)
    b2 = b2.astype(bf)
    in_maps = []
    for c in range(NCORES):
        g, r = divmod(c, 4)
        in_maps.append({
            "x": np.ascontiguousarray(x[g, r * TOK:(r + 1) * TOK, :]),
            "wq": wq, "wk": wk, "wv": wv, "wo": wo,
            "w1": w1, "w2": w2, "b1": b1, "b2": b2,
            "eye": np.eye(128, dtype=bf),
        })
    return in_maps


def kernel(**inputs):
    if "nc" not in _cache:
        _cache["nc"] = _build()
    from concourse.bass_utils import run_bass_kernel_spmd
    in_maps = _prepare_in_maps(inputs)
    res = run_bass_kernel_spmd(_cache["nc"], in_maps, core_ids=list(range(NCORES)))
    out = np.empty((B, S, H), np.float32)
    for c in range(NCORES):
        g, r = divmod(c, 4)
        out[g, r * TOK:(r + 1) * TOK, :] = res.results[c]["out"]
    return out
